# revision 1
# baseline (speedup 1.0000x reference)
"""GATv2 Bass kernel for Trainium2, 8 NeuronCores.

Problem: B=2, N=512, FIN=128, H=4, D=64 GATv2 attention (dense graph).
Sharding: one (batch, head) pair per core (B*H = 8 = n_cores).

Math per (b, h) (reference):
  h[n, d]  = x[n, :] @ W_proj[h]  (+ b_proj[h])
  zi[n, e] = h[n, :] @ W1[h, e, :]      (W1 = W_cat_weight[:, :, :D])
  zj[n, e] = h[n, :] @ W2[h, e, :]      (W2 = W_cat_weight[:, :, D:])
  score[i, j] = sum_e a[h, e] * lrelu(zi[i, e] + zj[j, e] + bcat[h, e])
  attn = softmax_j(score);  out[i, :] = attn[i, :] @ h  (+ bias_param slice)

Kernel decomposition (lrelu(v) = 0.6 v + 0.4 |v|):
  score[i,j] = A_i + B_j + sum_e sign(a_e) * |u[i,j,e]|,
    u = 0.4|a_e| (zi + zj + bcat)   (0.4|a| folded into W1/W2/bcat host-side)
  A_i is constant per row -> drops out of softmax_j. B_j is rank-1, done
  exactly in fp32 via a small matmul, replicated over the i partitions.
  Per core the e-columns are permuted positives-first; the global reduce
  ranges are the sorted unique k+ boundaries across all heads (so the one
  SPMD program fits every core), and each core supplies +-1 range signs
  that ScalarE applies as per-partition scales; GpSimd sums the ranges.
  V[i, (j, e)] = zi''[i, e] + Y''[j, e] is produced by TensorE as ONE
  bf16 matmul per full 512-wide PSUM bank: stationary lhsT = [zi''^T
  rows; zero pad; ones row], moving rhs = [tiled identity; zero pad;
  flattened Y'' row].  VectorE reduces |V| straight out of PSUM with
  tensor_reduce(apply_absolute_value=True) over each range.
"""

import os
import numpy as np
import ml_dtypes

import concourse.bacc as bacc
import concourse.mybir as mybir
import concourse.tile as tile
from concourse.bass_utils import run_bass_kernel_spmd

F32 = mybir.dt.float32
BF16 = mybir.dt.bfloat16
BF = ml_dtypes.bfloat16

B, N, FIN, H, D = 2, 512, 128, 4, 64
NEG_SLOPE = 0.2
C_LIN = (1.0 + NEG_SLOPE) / 2.0   # 0.6
C_ABS = (1.0 - NEG_SLOPE) / 2.0   # 0.4

NB = N // 128             # 4 row blocks of 128
E = D                     # e-dim width (64, unpadded)
J_PB = 512 // E           # j columns per PSUM bank (full 512-elem banks)
N_BANKS = N // J_PB       # banks per i-block
VT_BANKS = 4              # banks per V psum tile
N_VT = N_BANKS // VT_BANKS
VT_J = J_PB * VT_BANKS    # j columns per V tile
IB_N = min(16, N_BANKS)   # identity banks provided via DRAM

last_results = None        # BassKernelResults of the most recent run

_cache = {}


def _build(ranges, use_bcat, use_bproj, use_bias_param):
    """Build + compile the SPMD Bass program. All cores run this one NEFF."""
    nc = bacc.Bacc("TRN2", target_bir_lowering=False, debug=False, num_devices=8)

    nr = len(ranges)
    bounds = np.concatenate([[0], np.cumsum(ranges)]).astype(int)
    assert bounds[-1] == E

    x_d = nc.dram_tensor("x", [N, FIN], F32, kind="ExternalInput")
    wp_d = nc.dram_tensor("wproj", [FIN, D], F32, kind="ExternalInput")
    w1t_d = nc.dram_tensor("w1t", [FIN, 128], F32, kind="ExternalInput")
    zib_d = nc.dram_tensor("zib", [128, 1], F32, kind="ExternalInput")
    w2t_d = nc.dram_tensor("w2t", [FIN, E], F32, kind="ExternalInput")
    s15_d = nc.dram_tensor("s15", [E, 128], F32, kind="ExternalInput")
    id16_d = nc.dram_tensor("ident16", [E, IB_N * 512], BF16, kind="ExternalInput")
    id128_d = nc.dram_tensor("id128", [128, 128], F32, kind="ExternalInput")
    id128b_d = nc.dram_tensor("id128b", [128, 128], BF16, kind="ExternalInput")
    grs_d = nc.dram_tensor("grs", [128, nr], F32, kind="ExternalInput")
    if use_bcat:
        bc_d = nc.dram_tensor("bcat", [E, 1], F32, kind="ExternalInput")
    # (bcat input now carries bcat'' + W2''@b_proj; enabled if either nonzero)
    if use_bproj:
        bp_d = nc.dram_tensor("bproj", [D, 1], F32, kind="ExternalInput")
        bpr_d = nc.dram_tensor("bprojrep", [128, D], F32, kind="ExternalInput")
    if use_bias_param:
        bprm_d = nc.dram_tensor("biasprm", [128, D], F32, kind="ExternalInput")
    out_d = nc.dram_tensor("out", [N, D], F32, kind="ExternalOutput")

    AF = mybir.ActivationFunctionType
    ALU = mybir.AluOpType
    AX = mybir.AxisListType

    with tile.TileContext(nc) as tc:
        with tc.tile_pool(name="sb", bufs=1) as sb:
            # ---------- persistent SBUF tiles ----------
            xb = sb.tile([128, NB * 128], F32)
            xT = sb.tile([128, N], F32)
            wp = sb.tile([FIN, D], F32)
            w1t = sb.tile([FIN, 128], F32)
            zib = sb.tile([128, 1], F32)
            w2t = sb.tile([FIN, E], F32)
            s15 = sb.tile([E, 128], F32)
            id128 = sb.tile([128, 128], F32)
            id128b = sb.tile([128, 128], BF16)
            grs = sb.tile([128, nr], F32)
            h_hi = sb.tile([128, NB * D], BF16)
            h_lo = sb.tile([128, NB * D], BF16)
            ziT = sb.tile([128, N], BF16)             # rows 0:E zi''; 127 ones
            yTf = sb.tile([E, N], F32)
            yTb = sb.tile([E, N], BF16)
            ysb = sb.tile([128, NB * E], BF16)
            rhs_mega = sb.tile([128, N_BANKS * 512], BF16)
            B_sb = sb.tile([128, N], F32)
            R = [[sb.tile([128, N], F32, tag=f"R{i}_{r}", name=f"R{i}_{r}")
                  for r in range(nr)] for i in range(NB)]
            sc = [sb.tile([128, N], F32, tag=f"sc{i}", name=f"sc{i}")
                  for i in range(NB)]
            tg = [sb.tile([128, N], F32, tag=f"tg{i}", name=f"tg{i}")
                  for i in range(NB)]
            ee = [sb.tile([128, N], BF16, tag=f"ee{i}", name=f"ee{i}")
                  for i in range(NB)]
            rowmax = [sb.tile([128, 1], F32, tag=f"rm{i}", name=f"rm{i}")
                      for i in range(NB)]
            negm = [sb.tile([128, 1], F32, tag=f"nm{i}", name=f"nm{i}")
                    for i in range(NB)]
            zsum = [sb.tile([128, 1], F32, tag=f"zs{i}", name=f"zs{i}")
                    for i in range(NB)]
            rz = [sb.tile([128, 1], F32, tag=f"rz{i}", name=f"rz{i}")
                  for i in range(NB)]
            if use_bcat:
                bc = sb.tile([E, 1], F32)
            if use_bproj:
                bp = sb.tile([D, 1], F32)
                bpr = sb.tile([128, D], F32)
            if use_bias_param:
                bprm = sb.tile([128, D], F32)

            # ---------- input DMAs ----------
            for nb in range(NB):
                nc.sync.dma_start(
                    xb[:, nb * 128:(nb + 1) * 128],
                    x_d.ap()[nb * 128:(nb + 1) * 128, :])
            nc.sync.dma_start(wp[:], wp_d.ap())
            nc.sync.dma_start(w1t[:], w1t_d.ap())
            nc.sync.dma_start(zib[:], zib_d.ap())
            nc.sync.dma_start(w2t[:], w2t_d.ap())
            nc.sync.dma_start(s15[:], s15_d.ap())
            nc.sync.dma_start(id128[:], id128_d.ap())
            nc.sync.dma_start(id128b[:], id128b_d.ap())
            nc.sync.dma_start(grs[:], grs_d.ap())
            if use_bcat:
                nc.sync.dma_start(bc[:], bc_d.ap())
            if use_bproj:
                nc.sync.dma_start(bp[:], bp_d.ap())
                nc.sync.dma_start(bpr[:], bpr_d.ap())
            if use_bias_param:
                nc.sync.dma_start(bprm[:], bprm_d.ap())

            # rhs_mega: zero pad rows; identity region first IB_N banks via DMA
            nc.scalar.memzero(rhs_mega[64:128, 0:IB_N * 512])
            nc.sync.dma_start(rhs_mega[0:E, 0:IB_N * 512], id16_d.ap())

            # ---------- prep phase ----------
            with tc.tile_pool(name="pp", bufs=4, space="PSUM") as pp:
                for nb in range(NB):
                    t = pp.tile([128, 512], F32, tag="t")
                    nc.tensor.transpose(t[:, 0:128], xb[:, nb * 128:(nb + 1) * 128],
                                        id128[:])
                    nc.scalar.copy(xT[:, nb * 128:(nb + 1) * 128], t[:, 0:128])
                # zi''^T / Y''^T blockwise so the main loop can start early
                for nb in range(NB):
                    s_ = slice(nb * 128, (nb + 1) * 128)
                    t = pp.tile([128, 512], F32, tag="t")
                    nc.tensor.matmul(t[:, 0:128], w1t[:], xT[:, s_])
                    nc.scalar.activation(ziT[:, s_], t[:, 0:128], AF.Identity,
                                         bias=zib[:, 0:1])
                    t2 = pp.tile([128, 512], F32, tag="t")
                    nc.tensor.matmul(t2[0:E, 0:128], w2t[:], xT[:, s_])
                    if use_bcat:
                        nc.scalar.activation(yTf[:, s_], t2[0:E, 0:128],
                                             AF.Identity, bias=bc[:, 0:1])
                    else:
                        nc.scalar.copy(yTf[:, s_], t2[0:E, 0:128])
                    nc.scalar.copy(yTb[:, s_], yTf[:, s_])
                    tb = pp.tile([128, 512], BF16, tag="tb")
                    nc.tensor.transpose(tb[:, 0:E], yTb[:, s_], id128b[0:E, 0:E])
                    nc.scalar.copy(ysb[:, nb * E:(nb + 1) * E], tb[:, 0:E])
                    dst = rhs_mega[127:128, :].rearrange("o (n e) -> o n e", e=E)
                    nc.sync.dma_start(
                        dst[:, nb * 128:(nb + 1) * 128, :],
                        ysb[:, nb * E:(nb + 1) * E])
                # B_j replicated over i
                t = pp.tile([128, N], F32, tag="t")
                nc.tensor.matmul(t[:], s15[:], yTf[:])
                nc.scalar.copy(B_sb[:], t[:])
                # h (n x d) row blocks (off the critical path; agg only)
                for nb in range(NB):
                    t = pp.tile([128, 512], F32, tag="t")
                    nc.tensor.matmul(t[:, 0:D], xT[:, nb * 128:(nb + 1) * 128], wp[:])
                    if use_bproj:
                        nc.vector.tensor_tensor(t[:, 0:D], t[:, 0:D], bpr[:],
                                                op=ALU.add)
                    nc.scalar.copy(h_hi[:, nb * D:(nb + 1) * D], t[:, 0:D])
                    nc.vector.tensor_tensor(
                        h_lo[:, nb * D:(nb + 1) * D], t[:, 0:D],
                        h_hi[:, nb * D:(nb + 1) * D], op=ALU.subtract)

            # identity doubling in chunks on ScalarE (after prep emission so
            # the prep ACT ops run first in its FIFO)
            m = IB_N
            while m < N_BANKS:
                c = min(IB_N // 2, N_BANKS - m)
                nc.scalar.copy(
                    rhs_mega[0:127, m * 512:(m + c) * 512],
                    rhs_mega[0:127, 0:c * 512])
                m += c

            # ---------- main loop ----------
            deferred = []
            with tc.tile_pool(name="vp", bufs=2, space="PSUM") as vp:
                for ib in range(NB):
                    zi_l = ziT[:, ib * 128:(ib + 1) * 128]
                    for vt in range(N_VT):
                        if vt == 3 and deferred:
                            deferred.pop(0)()
                        v = vp.tile([128, VT_BANKS * 512], F32, tag="v")
                        for k in range(VT_BANKS):
                            bb = vt * VT_BANKS + k
                            nc.tensor.matmul(
                                v[:, k * 512:(k + 1) * 512], zi_l,
                                rhs_mega[:, bb * 512:(bb + 1) * 512],
                                start=True, stop=True)
                        v4 = v[:, :].rearrange("p (j e) -> p j e", e=E)
                        for r in range(nr):
                            ro = R[ib][r][:, vt * VT_J:(vt + 1) * VT_J]
                            nc.vector.tensor_reduce(
                                ro, v4[:, :, int(bounds[r]):int(bounds[r + 1])],
                                axis=AX.X, op=ALU.add,
                                apply_absolute_value=True)
                    # combine: sc = sum_r g_r * R_r + B
                    # (ScalarE mults; adds on GpSimd, except last ib on DVE
                    #  which is idle by then)
                    add_eng = nc.vector if ib == NB - 1 else nc.gpsimd
                    nc.scalar.activation(tg[ib][:], R[ib][0][:], AF.Copy, bias=0.0,
                                         scale=grs[:, 0:1])
                    for r in range(1, nr):
                        nc.scalar.activation(R[ib][r][:], R[ib][r][:], AF.Copy,
                                             bias=0.0, scale=grs[:, r:r + 1])
                    first = R[ib][1][:] if nr > 1 else B_sb[:]
                    add_eng.tensor_tensor(sc[ib][:], tg[ib][:], first, op=ALU.add)
                    for r in range(2, nr):
                        add_eng.tensor_tensor(sc[ib][:], sc[ib][:], R[ib][r][:],
                                              op=ALU.add)
                    if nr > 1:
                        add_eng.tensor_tensor(sc[ib][:], sc[ib][:], B_sb[:],
                                              op=ALU.add)
                    # softmax pieces: the DVE ops wait on the GpSimd combine,
                    # so defer their emission into the next i-block's stream
                    # (keeps the strict-FIFO DVE queue from stalling)
                    def _softmax(ib=ib):
                        nc.vector.tensor_reduce(rowmax[ib][:], sc[ib][:],
                                                axis=AX.X, op=ALU.max)
                        nc.vector.tensor_scalar_mul(negm[ib][:], rowmax[ib][:],
                                                    -1.0)
                        nc.scalar.activation(ee[ib][:], sc[ib][:], AF.Exp,
                                             bias=negm[ib][:, 0:1],
                                             accum_out=zsum[ib][:])
                        nc.vector.reciprocal(rz[ib][:], zsum[ib][:])
                    if ib < NB - 1:
                        deferred.append(_softmax)
                    else:
                        _softmax()
                while deferred:
                    deferred.pop(0)()

            # ---------- epilogue: attn @ h ----------
            with tc.tile_pool(name="ep", bufs=2, space="PSUM") as ep:
                for ib in range(NB):
                    eT = sb.tile([128, 128 * NB], BF16, tag=f"eT{ib}",
                                 name=f"eT{ib}")
                    for jb in range(NB):
                        t = ep.tile([128, 128], BF16, tag="et")
                        nc.tensor.transpose(
                            t[:], ee[ib][:, jb * 128:(jb + 1) * 128], id128b[:])
                        nc.scalar.copy(eT[:, jb * 128:(jb + 1) * 128], t[:])
                    acc = ep.tile([128, D], F32, tag="acc")
                    for jb in range(NB):
                        nc.tensor.matmul(
                            acc[:], eT[:, jb * 128:(jb + 1) * 128],
                            h_hi[:, jb * D:(jb + 1) * D],
                            start=(jb == 0), stop=False)
                        nc.tensor.matmul(
                            acc[:], eT[:, jb * 128:(jb + 1) * 128],
                            h_lo[:, jb * D:(jb + 1) * D],
                            start=False, stop=(jb == NB - 1))
                    o = sb.tile([128, D], F32, tag=f"o{ib}", name=f"o{ib}")
                    nc.scalar.activation(o[:], acc[:], AF.Copy, bias=0.0,
                                         scale=rz[ib][:, 0:1])
                    if use_bias_param:
                        nc.gpsimd.tensor_tensor(o[:], o[:], bprm[:], op=ALU.add)
                    nc.sync.dma_start(out_d.ap()[ib * 128:(ib + 1) * 128, :], o[:])

    nc.compile()
    return nc


def kernel(x, W_proj, b_proj, W_cat_weight, W_cat_bias, a, bias_param):
    global last_results
    x = np.asarray(x, dtype=np.float32)
    W_proj = np.asarray(W_proj, dtype=np.float32)
    b_proj = np.asarray(b_proj, dtype=np.float32)
    W_cat_weight = np.asarray(W_cat_weight, dtype=np.float32)
    W_cat_bias = np.asarray(W_cat_bias, dtype=np.float32)
    a = np.asarray(a, dtype=np.float32)
    bias_param = np.asarray(bias_param, dtype=np.float32)

    W1 = W_cat_weight[:, :, :D]
    W2 = W_cat_weight[:, :, D:]

    kpos = [int((a[h] > 0).sum()) for h in range(H)]
    pts = sorted({k for k in kpos if 0 < k < E})
    widths = tuple(int(w) for w in np.diff([0] + pts + [E]) if w > 0) or (E,)

    use_bcat = bool(np.any(W_cat_bias)) or bool(np.any(b_proj))
    use_bproj = bool(np.any(b_proj))
    use_bias_param = bool(np.any(bias_param))

    key = (widths, use_bcat, use_bproj, use_bias_param)
    if key not in _cache:
        _cache[key] = _build(*key)
    nc = _cache[key]

    nr = len(widths)
    bounds = np.concatenate([[0], np.cumsum(widths)]).astype(int)
    ident16 = np.tile(np.eye(E, dtype=np.float32), (1, IB_N * J_PB)).astype(BF)
    id128 = np.eye(128, dtype=np.float32)

    in_maps = []
    for c in range(8):
        b, h = divmod(c, H)
        ah = a[h]
        pos = np.where(ah > 0)[0]
        neg = np.where(ah <= 0)[0]
        kp = len(pos)
        slots = np.concatenate([pos, neg])
        scale = C_ABS * np.abs(ah[slots])        # 0.4|a| per slot
        sgn = np.sign(ah[slots])
        W1p = np.zeros((128, D), dtype=np.float32)
        W2p = np.zeros((E, D), dtype=np.float32)
        W1p[:E] = W1[h][slots] * scale[:, None]
        W2p[:] = W2[h][slots] * scale[:, None]
        M1 = W1p @ W_proj[h].T            # (128, FIN): zi'' from x^T directly
        M2 = W2p @ W_proj[h].T            # (E, FIN)
        s15 = np.tile((1.5 * sgn)[:, None], (1, 128)).astype(np.float32)
        zibv = np.zeros((128, 1), dtype=np.float32)
        zibv[127, 0] = 1.0
        zibv[:E, 0] = W1p[:E] @ b_proj[h]
        # range r is all-positive for this core iff its end <= kp (or kp == E)
        g = np.array([1.0 if (bounds[r + 1] <= kp or kp == E) else -1.0
                      for r in range(nr)], dtype=np.float32)
        grs = np.tile(g[None, :], (128, 1)).astype(np.float32)
        m = {
            "x": np.ascontiguousarray(x[b]),
            "wproj": np.ascontiguousarray(W_proj[h]),
            "w1t": np.ascontiguousarray(M1.T),
            "zib": zibv,
            "w2t": np.ascontiguousarray(M2.T),
            "s15": s15,
            "ident16": ident16,
            "id128": id128,
            "id128b": id128.astype(BF),
            "grs": grs,
        }
        if use_bcat:
            m["bcat"] = ((W_cat_bias[h][slots] * scale) + W2p @ b_proj[h]
                         )[:, None].astype(np.float32)
        if use_bproj:
            m["bproj"] = np.ascontiguousarray(b_proj[h][:, None])
            m["bprojrep"] = np.tile(b_proj[h][None, :], (128, 1)).astype(np.float32)
        if use_bias_param:
            m["biasprm"] = np.tile(bias_param[None, h * D:(h + 1) * D],
                                   (128, 1)).astype(np.float32)
        in_maps.append(m)

    res = run_bass_kernel_spmd(nc, in_maps, core_ids=list(range(8)))
    last_results = res

    out = np.empty((B, N, H * D), dtype=np.float32)
    for c in range(8):
        b, h = divmod(c, H)
        out[b, :, h * D:(h + 1) * D] = res.results[c]["out"]
    return out



# revision 2
# speedup vs baseline: 1.0162x; 1.0162x over previous
"""GATv2 Bass kernel v2 for Trainium2, 8 NeuronCores.

Problem: B=2, N=512, FIN=128, H=4, D=64 GATv2 attention (dense graph).
Sharding: one (batch, head) pair per core (B*H = 8 = n_cores).

Math per (b, h):
  h[n] = x[n] @ Wp + bp
  zi[n,e] = h[n]·W1[e]; zj[n,e] = h[n]·W2[e]; v_ije = zi[i,e]+zj[j,e]+bc[e]
  score[i,j] = sum_e a_e lrelu(v); attn = softmax_j; out = attn @ h

ReLU decomposition (v2): lrelu(v) = v + 0.8 ReLU(-v), so
  score[i,j] = A_i + B_j + sum_e s_e ReLU(u_i(e) + y_j(e))
  u_i(e) = -0.8|a_e| zi[i,e] (+bias), y_j(e) = -0.8|a_e| (zj[j,e]+bc_e)
  s_e = sign(a_e); A_i constant per row -> drops in softmax.
Kernel: e lives on PARTITIONS. Ydup [128,512] = yT stacked twice (bf16).
Per row-pair (2 rows per 128 partitions): W = ReLU(Ydup + u-col) via ONE
fused DVE tensor_scalar (add,max) at 2-4x rate, or ACT activation(Relu,
bias). PE contracts with +-1 stationaries [128,32] (16 pairs accumulate
into one 32-partition stripe of the score bank; tile_position picks the
stripe). B_j added exactly via a [2,128]x[2,512] accumulating matmul with
B split hi/lo in bf16. Softmax reads scores straight from PSUM; epilogue
(attn @ h with h split hi/lo bf16) as in v1. Softmax/epilogue emission is
deferred by one i-block so the strict-FIFO DVE/ACT queues never stall.
"""

import numpy as np
import ml_dtypes

import concourse.bacc as bacc
import concourse.mybir as mybir
import concourse.tile as tile
from concourse.bass_utils import run_bass_kernel_spmd

F32 = mybir.dt.float32
BF16 = mybir.dt.bfloat16
BF = ml_dtypes.bfloat16

B, N, FIN, H, D = 2, 512, 128, 4, 64
NEG_SLOPE = 0.2
E = D
NB = N // 128

last_results = None

_cache = {}


def _build(use_bias_param):
    nc = bacc.Bacc("TRN2", target_bir_lowering=False, debug=False,
                   num_devices=8)

    x_d = nc.dram_tensor("x", [N, FIN], F32, kind="ExternalInput")
    id128_d = nc.dram_tensor("id128", [128, 128], F32, kind="ExternalInput")
    mm_d = nc.dram_tensor("mm", [128, 2 * E], BF16, kind="ExternalInput")
    pks_d = nc.dram_tensor("pks", [128, 3], F32, kind="ExternalInput")
    wp_d = nc.dram_tensor("wproj", [FIN, D], F32, kind="ExternalInput")
    s16_d = nc.dram_tensor("s16", [128, 512], BF16, kind="ExternalInput")
    id128b_d = nc.dram_tensor("id128b", [128, 128], BF16,
                              kind="ExternalInput")
    if use_bias_param:
        bprm_d = nc.dram_tensor("biasprm", [128, D], F32,
                                kind="ExternalInput")
    out_d = nc.dram_tensor("out", [N, D], F32, kind="ExternalOutput")

    AF = mybir.ActivationFunctionType
    ALU = mybir.AluOpType
    AX = mybir.AxisListType

    with tile.TileContext(nc) as tc:
        with tc.tile_pool(name="sb", bufs=1) as sb:
            xb = sb.tile([128, NB * 128], F32)
            xT = sb.tile([128, N], F32)
            xTb = sb.tile([128, N], BF16)
            id128 = sb.tile([128, 128], F32)
            mmt = sb.tile([128, 2 * E], BF16)
            pks = sb.tile([128, 3], F32)
            sBb = sb.tile([E, 1], BF16)
            s16 = sb.tile([128, 512], BF16)
            wp = sb.tile([FIN, D], F32)
            id128b = sb.tile([128, 128], BF16)
            ydup = sb.tile([128, N], BF16)
            zidup = sb.tile([128, NB * 64], F32)
            yTsb = sb.tile([E, N], BF16)
            b2hi = sb.tile([1, N], BF16)
            b2lo = sb.tile([1, N], BF16)
            ones1 = sb.tile([1, 128], BF16)
            h_hi = sb.tile([128, NB * D], BF16)
            h_lo = sb.tile([128, NB * D], BF16)
            h_f = sb.tile([128, NB * D], F32)
            bpf = sb.tile([1, N], F32)
            ee = [sb.tile([128, N], BF16, tag=f"ee{i}", name=f"ee{i}")
                  for i in range(NB)]
            rowmax = [sb.tile([128, 1], F32, tag=f"rm{i}", name=f"rm{i}")
                      for i in range(NB)]
            negm = [sb.tile([128, 1], F32, tag=f"nm{i}", name=f"nm{i}")
                    for i in range(NB)]
            zsum = [sb.tile([128, 1], F32, tag=f"zs{i}", name=f"zs{i}")
                    for i in range(NB)]
            rz = [sb.tile([128, 1], F32, tag=f"rz{i}", name=f"rz{i}")
                  for i in range(NB)]
            if use_bias_param:
                bprm = sb.tile([128, D], F32)

            # ---------- input DMAs: spread across engine DGEs ----------
            for nb in range(NB):
                nc.sync.dma_start(
                    xb[:, nb * 128:(nb + 1) * 128],
                    x_d.ap()[nb * 128:(nb + 1) * 128, :])
            nc.scalar.dma_start(id128[:, 0:64], id128_d.ap()[:, 0:64])
            nc.scalar.dma_start(id128[:, 64:128], id128_d.ap()[:, 64:128])
            nc.gpsimd.dma_start(s16[:, 0:256], s16_d.ap()[:, 0:256])
            nc.gpsimd.dma_start(s16[:, 256:512], s16_d.ap()[:, 256:512])
            nc.scalar.dma_start(mmt[:], mm_d.ap())
            nc.scalar.dma_start(pks[:], pks_d.ap())
            nc.gpsimd.dma_start(wp[:], wp_d.ap())
            nc.gpsimd.dma_start(id128b[:], id128b_d.ap())
            if use_bias_param:
                nc.scalar.dma_start(bprm[:], bprm_d.ap())

            nc.gpsimd.memset(ones1[:], 1.0)

            # ---------- prep ----------
            with tc.tile_pool(name="pp", bufs=4, space="PSUM") as pp:
                for nb in range(NB):
                    t = pp.tile([128, 512], F32, tag="t")
                    nc.tensor.transpose(t[:, 0:128],
                                        xb[:, nb * 128:(nb + 1) * 128],
                                        id128[:])
                    nc.scalar.copy(xT[:, nb * 128:(nb + 1) * 128],
                                   t[:, 0:128])
                    nc.vector.tensor_copy(xTb[:, nb * 128:(nb + 1) * 128],
                                          t[:, 0:128])
                nc.scalar.copy(sBb[:], pks[0:E, 2:3])
                # yT = M2 @ xT + c2 (bf16 mm) FIRST: it gates ydup -> maps
                yt = pp.tile([128, 512], F32, tag="t")
                nc.tensor.matmul(yt[0:E, :], mmt[:, E:2 * E], xTb[:],
                                 start=True, stop=True)
                # uT = M1 @ xT + c1 (bf16 mm), scattered into zidup (f32)
                ut = pp.tile([128, 512], F32, tag="t")
                nc.tensor.matmul(ut[0:E, :], mmt[:, 0:E], xTb[:],
                                 start=True, stop=True)
                # DVE: zidup scatter (fused add of c1) straight from PSUM
                utv = ut[0:E, :].rearrange("p (b q t) -> p b q t",
                                           b=NB, q=64, t=2)
                zde = zidup[0:E, :].rearrange("p (b q t) -> p b q t",
                                              b=NB, q=64, t=1)
                zdo = zidup[E:128, :].rearrange("p (b q t) -> p b q t",
                                                b=NB, q=64, t=1)
                nc.vector.tensor_scalar(zde[:, :, :, :], utv[:, :, :, 0:1],
                                        pks[0:E, 0:1], None, op0=ALU.add)
                nc.vector.tensor_scalar(zdo[:, :, :, :], utv[:, :, :, 1:2],
                                        pks[0:E, 0:1], None, op0=ALU.add)
                # ACT: ydup (gates the relu maps), then yTsb / b2
                nc.scalar.activation(ydup[0:E, :], yt[0:E, :], AF.Identity,
                                     bias=pks[0:E, 1:2])
                nc.scalar.activation(ydup[E:128, :], yt[0:E, :], AF.Identity,
                                     bias=pks[0:E, 1:2])
                nc.scalar.activation(yTsb[:], yt[0:E, :], AF.Identity,
                                     bias=pks[0:E, 1:2])

            # ---------- main: per i-block scores, softmax/epilogue of the
            # previous block interleaved at the HEAD of each block so the
            # strict-FIFO ACT/DVE queues never sit behind a full block of
            # relu maps ----------
            banks = []
            accs = {}
            eTs = {}
            hts = []
            bps = []

            def emit_map(ib, q, on_act):
                w = wpool.tile([128, N], BF16, tag="w")
                col = ib * 64 + q
                if on_act:
                    with nc.allow_low_precision(reason="bf16 relu"):
                        nc.scalar.activation(w[:], ydup[:], AF.Relu,
                                             bias=zidup[:, col:col + 1])
                else:
                    with nc.allow_low_precision(reason="bf16 relu"):
                        nc.vector.tensor_scalar(
                            w[:], ydup[:], zidup[:, col:col + 1],
                            0.0, op0=ALU.add, op1=ALU.max)
                st, m = divmod(q, 16)
                nc.tensor.matmul(
                    bank[32 * st:32 * st + 32, :],
                    s16[:, 32 * m:32 * m + 32], w[:],
                    start=(m == 0), stop=False,
                    tile_position=(0, 32 * st),
                    skip_group_check=True)

            def act_map(q):
                # ACT handles every 4th map from q=15 on; the head of each
                # block is DVE-only so ACT can run the previous block's
                # exp/epilogue without stalling the PE map stream
                return q >= 12 and q % 4 == 3

            def emit_block(ib, prev):
                global bank
                bank = scp.tile([128, N], F32, tag="bank", name=f"bank{ib}")
                banks.append(bank)
                for q in range(64):
                    emit_map(ib, q, act_map(q))
                    if prev is None:
                        if q == 4:
                            hall = epp.tile([128, NB * D], F32, tag="hall",
                                            bufs=1)
                            hts.append(hall)
                            for nb in range(NB):
                                nc.tensor.matmul(
                                    hall[:, nb * D:(nb + 1) * D],
                                    xT[:, nb * 128:(nb + 1) * 128],
                                    wp[:], start=True, stop=True,
                                    skip_group_check=True)
                        elif q == 6:
                            for nb in range(NB):
                                nc.scalar.copy(h_hi[:, nb * D:(nb + 1) * D],
                                               hts[0][:, nb * D:(nb + 1) * D])
                                nc.scalar.copy(h_f[:, nb * D:(nb + 1) * D],
                                               hts[0][:, nb * D:(nb + 1) * D])
                        elif q == 8:
                            bp_ = epp.tile([128, 512], F32, tag="bp",
                                           bufs=1)
                            bps.append(bp_)
                            nc.tensor.matmul(bp_[0:1, :], sBb[:], yTsb[:],
                                             start=True, stop=True)
                        elif q == 10:
                            nc.scalar.copy(b2hi[:], bps[0][0:1, :])
                            nc.scalar.copy(bpf[:], bps[0][0:1, :])
                        elif q == 24:
                            for nb in range(NB):
                                nc.vector.tensor_tensor(
                                    h_lo[:, nb * D:(nb + 1) * D],
                                    h_f[:, nb * D:(nb + 1) * D],
                                    h_hi[:, nb * D:(nb + 1) * D],
                                    op=ALU.subtract)
                        elif q == 40:
                            nc.vector.tensor_tensor(b2lo[:], bpf[:],
                                                    b2hi[:],
                                                    op=ALU.subtract)
                        continue
                    if q == 7:
                        # ACT exp for prev block (uncentered: scores bounded)
                        pb = banks[prev]
                        nc.scalar.activation(ee[prev][:], pb[:, :], AF.Exp,
                                             bias=0.0,
                                             accum_out=zsum[prev][:])
                        nc.vector.reciprocal(rz[prev][:], zsum[prev][:])
                    elif q == 9:
                        eT = sb.tile([128, 128 * NB], BF16, tag=f"eT{prev}",
                                     name=f"eT{prev}")
                        eTs[prev] = eT
                        for jb in range(NB):
                            t = epp.tile([128, 128], BF16, tag="et")
                            nc.tensor.transpose(
                                t[:], ee[prev][:, jb * 128:(jb + 1) * 128],
                                id128b[:])
                            nc.scalar.copy(
                                eT[:, jb * 128:(jb + 1) * 128], t[:])
                    elif q == 13:
                        eT = eTs[prev]
                        acc = epp.tile([128, D], F32, tag="acc",
                                       name=f"acc{prev}", bufs=1)
                        accs[prev] = acc
                        for jb in range(NB):
                            nc.tensor.matmul(
                                acc[:], eT[:, jb * 128:(jb + 1) * 128],
                                h_hi[:, jb * D:(jb + 1) * D],
                                start=(jb == 0), stop=False)
                            nc.tensor.matmul(
                                acc[:], eT[:, jb * 128:(jb + 1) * 128],
                                h_lo[:, jb * D:(jb + 1) * D],
                                start=False, stop=(jb == NB - 1))
                    elif q == 17:
                        emit_out(prev)
                # exact B add (accumulate, close the bank)
                nc.tensor.matmul(bank[:, :], ones1[:], b2hi[:],
                                 start=False, stop=False,
                                 skip_group_check=True)
                nc.tensor.matmul(bank[:, :], ones1[:], b2lo[:],
                                 start=False, stop=True,
                                 skip_group_check=True)

            def emit_out(ib):
                o = sb.tile([128, D], F32, tag=f"o{ib}", name=f"o{ib}")
                nc.scalar.activation(o[:], accs[ib][:], AF.Copy, bias=0.0,
                                     scale=rz[ib][:, 0:1])
                if use_bias_param:
                    nc.vector.tensor_tensor(o[:], o[:], bprm[:], op=ALU.add)
                nc.gpsimd.dma_start(out_d.ap()[ib * 128:(ib + 1) * 128, :],
                                    o[:])

            with tc.tile_pool(name="scores", bufs=2, space="PSUM") as scp, \
                 tc.tile_pool(name="wpl", bufs=8) as wpool, \
                 tc.tile_pool(name="ep", bufs=2, space="PSUM") as epp:
                for ib in range(NB):
                    emit_block(ib, ib - 1 if ib >= 1 else None)
                # tail: last block's softmax + epilogue
                lb = NB - 1
                pb = banks[lb]
                nc.scalar.activation(ee[lb][:], pb[:, :], AF.Exp,
                                     bias=0.0, accum_out=zsum[lb][:])
                nc.vector.reciprocal(rz[lb][:], zsum[lb][:])
                eT = sb.tile([128, 128 * NB], BF16, tag=f"eT{lb}",
                             name=f"eT{lb}")
                for jb in range(NB):
                    t = epp.tile([128, 128], BF16, tag="et")
                    nc.tensor.transpose(
                        t[:], ee[lb][:, jb * 128:(jb + 1) * 128], id128b[:])
                    nc.scalar.copy(eT[:, jb * 128:(jb + 1) * 128], t[:])
                acc = epp.tile([128, D], F32, tag="acc", name=f"acc{lb}",
                               bufs=1)
                accs[lb] = acc
                for jb in range(NB):
                    nc.tensor.matmul(
                        acc[:], eT[:, jb * 128:(jb + 1) * 128],
                        h_hi[:, jb * D:(jb + 1) * D],
                        start=(jb == 0), stop=False)
                    nc.tensor.matmul(
                        acc[:], eT[:, jb * 128:(jb + 1) * 128],
                        h_lo[:, jb * D:(jb + 1) * D],
                        start=False, stop=(jb == NB - 1))
                emit_out(lb)

    nc.compile()
    return nc


def kernel(x, W_proj, b_proj, W_cat_weight, W_cat_bias, a, bias_param):
    global last_results
    x = np.asarray(x, dtype=np.float32)
    W_proj = np.asarray(W_proj, dtype=np.float32)
    b_proj = np.asarray(b_proj, dtype=np.float32)
    W_cat_weight = np.asarray(W_cat_weight, dtype=np.float32)
    W_cat_bias = np.asarray(W_cat_bias, dtype=np.float32)
    a = np.asarray(a, dtype=np.float32)
    bias_param = np.asarray(bias_param, dtype=np.float32)

    W1 = W_cat_weight[:, :, :D]
    W2 = W_cat_weight[:, :, D:]

    use_bias_param = bool(np.any(bias_param))
    key = (use_bias_param,)
    if key not in _cache:
        _cache[key] = _build(*key)
    nc = _cache[key]

    id128 = np.eye(128, dtype=np.float32)
    s16 = None  # per-core below

    in_maps = []
    for c in range(8):
        b, hh = divmod(c, H)
        ah = a[hh]
        s = np.sign(ah).astype(np.float32)
        abs_a = np.abs(ah)
        Wp = W_proj[hh]
        bp = b_proj[hh]
        bc = W_cat_bias[hh]
        M1 = -0.8 * (abs_a[:, None] * W1[hh]) @ Wp.T     # [E, FIN]
        c1 = -0.8 * abs_a * (W1[hh] @ bp)                # [E]
        M2 = -0.8 * (abs_a[:, None] * W2[hh]) @ Wp.T     # [E, FIN]
        c2 = -0.8 * abs_a * (W2[hh] @ bp + bc)           # [E]
        mmt = np.concatenate([M1.T, M2.T], axis=1)       # [FIN, 2E]
        pks = np.zeros((128, 3), dtype=np.float32)
        pks[0:E, 0] = c1
        pks[0:E, 1] = c2
        pks[0:E, 2] = -1.25 * s
        s16 = np.zeros((128, 512), dtype=np.float32)
        for m in range(16):
            s16[0:E, 32 * m + 2 * m] = s
            s16[E:128, 32 * m + 2 * m + 1] = s
        m = {
            "x": np.ascontiguousarray(x[b]),
            "id128": id128,
            "mm": np.ascontiguousarray(mmt).astype(BF),
            "pks": pks,
            "wproj": np.ascontiguousarray(Wp),
            "s16": s16.astype(BF),
            "id128b": id128.astype(BF),
        }
        if use_bias_param:
            m["biasprm"] = np.tile(bias_param[None, hh * D:(hh + 1) * D],
                                   (128, 1)).astype(np.float32)
        in_maps.append(m)

    res = run_bass_kernel_spmd(nc, in_maps, core_ids=list(range(8)))
    last_results = res

    out = np.empty((B, N, H * D), dtype=np.float32)
    for c in range(8):
        b, hh = divmod(c, H)
        out[b, :, hh * D:(hh + 1) * D] = res.results[c]["out"]
    return out


# revision 3
# speedup vs baseline: 1.0414x; 1.0248x over previous
"""GATv2 Bass kernel v2 for Trainium2, 8 NeuronCores.

Problem: B=2, N=512, FIN=128, H=4, D=64 GATv2 attention (dense graph).
Sharding: one (batch, head) pair per core (B*H = 8 = n_cores).

Math per (b, h):
  h[n] = x[n] @ Wp + bp
  zi[n,e] = h[n]·W1[e]; zj[n,e] = h[n]·W2[e]; v_ije = zi[i,e]+zj[j,e]+bc[e]
  score[i,j] = sum_e a_e lrelu(v); attn = softmax_j; out = attn @ h

ReLU decomposition (v2): lrelu(v) = v + 0.8 ReLU(-v), so
  score[i,j] = A_i + B_j + sum_e s_e ReLU(u_i(e) + y_j(e))
  u_i(e) = -0.8|a_e| zi[i,e] (+bias), y_j(e) = -0.8|a_e| (zj[j,e]+bc_e)
  s_e = sign(a_e); A_i constant per row -> drops in softmax.
Kernel: e lives on PARTITIONS. Ydup [128,512] = yT stacked twice (bf16).
Per row-pair (2 rows per 128 partitions): W = ReLU(Ydup + u-col) via ONE
fused DVE tensor_scalar (add,max) at 2-4x rate, or ACT activation(Relu,
bias). PE contracts with +-1 stationaries [128,32] (16 pairs accumulate
into one 32-partition stripe of the score bank; tile_position picks the
stripe). B_j added exactly via a [2,128]x[2,512] accumulating matmul with
B split hi/lo in bf16. Softmax reads scores straight from PSUM; epilogue
(attn @ h with h split hi/lo bf16) as in v1. Softmax/epilogue emission is
deferred by one i-block so the strict-FIFO DVE/ACT queues never stall.
"""

import numpy as np
import ml_dtypes

import concourse.bacc as bacc
import concourse.mybir as mybir
import concourse.tile as tile
from concourse.bass_utils import run_bass_kernel_spmd

F32 = mybir.dt.float32
BF16 = mybir.dt.bfloat16
BF = ml_dtypes.bfloat16

B, N, FIN, H, D = 2, 512, 128, 4, 64
NEG_SLOPE = 0.2
E = D
NB = N // 128

last_results = None

_cache = {}


def _build(use_bias_param):
    nc = bacc.Bacc("TRN2", target_bir_lowering=False, debug=False,
                   num_devices=8)

    x_d = nc.dram_tensor("x", [N, FIN], F32, kind="ExternalInput")
    id128_d = nc.dram_tensor("id128", [128, 128], F32, kind="ExternalInput")
    mm_d = nc.dram_tensor("mm", [128, 2 * E], BF16, kind="ExternalInput")
    pks_d = nc.dram_tensor("pks", [128, 3], F32, kind="ExternalInput")
    wp_d = nc.dram_tensor("wproj", [FIN, D], F32, kind="ExternalInput")
    s16_d = nc.dram_tensor("s16", [128, 512], BF16, kind="ExternalInput")
    id128b_d = nc.dram_tensor("id128b", [128, 128], BF16,
                              kind="ExternalInput")
    if use_bias_param:
        bprm_d = nc.dram_tensor("biasprm", [128, D], F32,
                                kind="ExternalInput")
    out_d = nc.dram_tensor("out", [N, D], F32, kind="ExternalOutput")

    AF = mybir.ActivationFunctionType
    ALU = mybir.AluOpType
    AX = mybir.AxisListType

    with tile.TileContext(nc) as tc:
        with tc.tile_pool(name="sb", bufs=1) as sb:
            xb = sb.tile([128, NB * 128], F32)
            xT = sb.tile([128, N], F32)
            xTb = sb.tile([128, N], BF16)
            id128 = sb.tile([128, 128], F32)
            mmt = sb.tile([128, 2 * E], BF16)
            pks = sb.tile([128, 3], F32)
            sBb = sb.tile([E, 1], BF16)
            s16 = sb.tile([128, 512], BF16)
            wp = sb.tile([FIN, D], F32)
            id128b = sb.tile([128, 128], BF16)
            ydup = sb.tile([128, N], BF16)
            zidup = sb.tile([128, NB * 64], F32)
            yTsb = sb.tile([E, N], BF16)
            b2hi = sb.tile([1, N], BF16)
            b2lo = sb.tile([1, N], BF16)
            ones1 = sb.tile([1, 128], BF16)
            h_hi = sb.tile([128, NB * D], BF16)
            h_lo = sb.tile([128, NB * D], BF16)
            h_f = sb.tile([128, NB * D], F32)
            bpf = sb.tile([1, N], F32)
            ee = [sb.tile([128, N], BF16, tag=f"ee{i}", name=f"ee{i}")
                  for i in range(NB)]
            rowmax = [sb.tile([128, 1], F32, tag=f"rm{i}", name=f"rm{i}")
                      for i in range(NB)]
            negm = [sb.tile([128, 1], F32, tag=f"nm{i}", name=f"nm{i}")
                    for i in range(NB)]
            zsum = [sb.tile([128, 1], F32, tag=f"zs{i}", name=f"zs{i}")
                    for i in range(NB)]
            rz = [sb.tile([128, 1], F32, tag=f"rz{i}", name=f"rz{i}")
                  for i in range(NB)]
            if use_bias_param:
                bprm = sb.tile([128, D], F32)

            # ---------- input DMAs: spread across engine DGEs ----------
            for nb in range(NB):
                nc.sync.dma_start(
                    xb[:, nb * 128:(nb + 1) * 128],
                    x_d.ap()[nb * 128:(nb + 1) * 128, :])
            nc.scalar.dma_start(id128[:, 0:64], id128_d.ap()[:, 0:64])
            nc.scalar.dma_start(id128[:, 64:128], id128_d.ap()[:, 64:128])
            nc.gpsimd.dma_start(s16[:, 0:256], s16_d.ap()[:, 0:256])
            nc.gpsimd.dma_start(s16[:, 256:512], s16_d.ap()[:, 256:512])
            nc.scalar.dma_start(mmt[:], mm_d.ap())
            nc.scalar.dma_start(pks[:], pks_d.ap())
            nc.gpsimd.dma_start(wp[:], wp_d.ap())
            nc.gpsimd.dma_start(id128b[:], id128b_d.ap())
            if use_bias_param:
                nc.scalar.dma_start(bprm[:], bprm_d.ap())

            nc.gpsimd.memset(ones1[:], 1.0)

            # ---------- prep ----------
            with tc.tile_pool(name="pp", bufs=4, space="PSUM") as pp:
                for nb in range(NB):
                    t = pp.tile([128, 512], F32, tag="t")
                    nc.tensor.transpose(t[:, 0:128],
                                        xb[:, nb * 128:(nb + 1) * 128],
                                        id128[:])
                    nc.scalar.copy(xT[:, nb * 128:(nb + 1) * 128],
                                   t[:, 0:128])
                    nc.vector.tensor_copy(xTb[:, nb * 128:(nb + 1) * 128],
                                          t[:, 0:128])
                nc.scalar.copy(sBb[:], pks[0:E, 2:3])
                # yT = M2 @ xT + c2 (bf16 mm) FIRST: it gates ydup -> maps
                yt = pp.tile([128, 512], F32, tag="t")
                nc.tensor.matmul(yt[0:E, :], mmt[:, E:2 * E], xTb[:],
                                 start=True, stop=True)
                # uT = M1 @ xT + c1 (bf16 mm), scattered into zidup (f32)
                ut = pp.tile([128, 512], F32, tag="t")
                nc.tensor.matmul(ut[0:E, :], mmt[:, 0:E], xTb[:],
                                 start=True, stop=True)
                # DVE: zidup scatter (fused add of c1) straight from PSUM
                utv = ut[0:E, :].rearrange("p (b q t) -> p b q t",
                                           b=NB, q=64, t=2)
                zde = zidup[0:E, :].rearrange("p (b q t) -> p b q t",
                                              b=NB, q=64, t=1)
                zdo = zidup[E:128, :].rearrange("p (b q t) -> p b q t",
                                                b=NB, q=64, t=1)
                nc.vector.tensor_scalar(zde[:, :, :, :], utv[:, :, :, 0:1],
                                        pks[0:E, 0:1], None, op0=ALU.add)
                nc.vector.tensor_scalar(zdo[:, :, :, :], utv[:, :, :, 1:2],
                                        pks[0:E, 0:1], None, op0=ALU.add)
                # ACT: ydup (gates the relu maps), then yTsb / b2
                nc.scalar.activation(ydup[0:E, :], yt[0:E, :], AF.Identity,
                                     bias=pks[0:E, 1:2])
                nc.scalar.activation(ydup[E:128, :], yt[0:E, :], AF.Identity,
                                     bias=pks[0:E, 1:2])
                nc.scalar.activation(yTsb[:], yt[0:E, :], AF.Identity,
                                     bias=pks[0:E, 1:2])

            # ---------- main: per i-block scores, softmax/epilogue of the
            # previous block interleaved at the HEAD of each block so the
            # strict-FIFO ACT/DVE queues never sit behind a full block of
            # relu maps ----------
            banks = []
            accs = {}
            eTs = {}
            hts = []
            bps = []

            def emit_map(ib, q, on_act):
                w = wpool.tile([128, N], BF16, tag="w")
                col = ib * 64 + q
                if on_act:
                    with nc.allow_low_precision(reason="bf16 relu"):
                        nc.scalar.activation(w[:], ydup[:], AF.Relu,
                                             bias=zidup[:, col:col + 1])
                else:
                    with nc.allow_low_precision(reason="bf16 relu"):
                        nc.vector.tensor_scalar(
                            w[:], ydup[:], zidup[:, col:col + 1],
                            0.0, op0=ALU.add, op1=ALU.max)
                st, m = divmod(q, 16)
                nc.tensor.matmul(
                    bank[32 * st:32 * st + 32, :],
                    s16[:, 32 * m:32 * m + 32], w[:],
                    start=(m == 0), stop=False,
                    tile_position=(0, 32 * st),
                    skip_group_check=True)

            def act_map(q):
                # ACT handles every 4th map from q=15 on; the head of each
                # block is DVE-only so ACT can run the previous block's
                # exp/epilogue without stalling the PE map stream
                return q >= 12 and q % 4 == 3

            def emit_block(ib, prev):
                global bank
                bank = scp.tile([128, N], F32, tag="bank", name=f"bank{ib}")
                banks.append(bank)
                for q in range(64):
                    emit_map(ib, q, act_map(q))
                    if prev is None:
                        if q == 4:
                            hall = epp.tile([128, NB * D], F32, tag="hall",
                                            bufs=1)
                            hts.append(hall)
                            for nb in range(NB):
                                nc.tensor.matmul(
                                    hall[:, nb * D:(nb + 1) * D],
                                    xT[:, nb * 128:(nb + 1) * 128],
                                    wp[:], start=True, stop=True,
                                    skip_group_check=True)
                        elif q == 6:
                            for nb in range(NB):
                                nc.scalar.copy(h_hi[:, nb * D:(nb + 1) * D],
                                               hts[0][:, nb * D:(nb + 1) * D])
                                nc.scalar.copy(h_f[:, nb * D:(nb + 1) * D],
                                               hts[0][:, nb * D:(nb + 1) * D])
                        elif q == 8:
                            bp_ = epp.tile([128, 512], F32, tag="bp",
                                           bufs=1)
                            bps.append(bp_)
                            nc.tensor.matmul(bp_[0:1, :], sBb[:], yTsb[:],
                                             start=True, stop=True)
                        elif q == 10:
                            nc.scalar.copy(b2hi[:], bps[0][0:1, :])
                            nc.scalar.copy(bpf[:], bps[0][0:1, :])
                        elif q == 24:
                            for nb in range(NB):
                                nc.vector.tensor_tensor(
                                    h_lo[:, nb * D:(nb + 1) * D],
                                    h_f[:, nb * D:(nb + 1) * D],
                                    h_hi[:, nb * D:(nb + 1) * D],
                                    op=ALU.subtract)
                        elif q == 40:
                            nc.vector.tensor_tensor(b2lo[:], bpf[:],
                                                    b2hi[:],
                                                    op=ALU.subtract)
                        continue
                    if q == 7:
                        # ACT exp for prev block (uncentered: scores bounded)
                        pb = banks[prev]
                        nc.scalar.activation(ee[prev][:], pb[:, :], AF.Exp,
                                             bias=0.0,
                                             accum_out=zsum[prev][:])
                        nc.vector.reciprocal(rz[prev][:], zsum[prev][:])
                    elif q == 9:
                        eT = sb.tile([128, 128 * NB], BF16, tag=f"eT{prev}",
                                     name=f"eT{prev}")
                        eTs[prev] = eT
                        for jb in range(NB):
                            t = epp.tile([128, 128], BF16, tag="et")
                            nc.tensor.transpose(
                                t[:], ee[prev][:, jb * 128:(jb + 1) * 128],
                                id128b[:])
                            nc.scalar.copy(
                                eT[:, jb * 128:(jb + 1) * 128], t[:])
                    elif q == 13:
                        eT = eTs[prev]
                        acc = epp.tile([128, D], F32, tag="acc",
                                       name=f"acc{prev}", bufs=1)
                        accs[prev] = acc
                        for jb in range(NB):
                            nc.tensor.matmul(
                                acc[:], eT[:, jb * 128:(jb + 1) * 128],
                                h_hi[:, jb * D:(jb + 1) * D],
                                start=(jb == 0), stop=False)
                            nc.tensor.matmul(
                                acc[:], eT[:, jb * 128:(jb + 1) * 128],
                                h_lo[:, jb * D:(jb + 1) * D],
                                start=False, stop=(jb == NB - 1))
                    elif q == 17:
                        emit_out(prev)
                # exact B add (accumulate, close the bank)
                nc.tensor.matmul(bank[:, :], ones1[:], b2hi[:],
                                 start=False, stop=False,
                                 skip_group_check=True)
                nc.tensor.matmul(bank[:, :], ones1[:], b2lo[:],
                                 start=False, stop=True,
                                 skip_group_check=True)

            def emit_out(ib):
                o = sb.tile([128, D], F32, tag=f"o{ib}", name=f"o{ib}")
                nc.scalar.activation(o[:], accs[ib][:], AF.Copy, bias=0.0,
                                     scale=rz[ib][:, 0:1])
                if use_bias_param:
                    nc.vector.tensor_tensor(o[:], o[:], bprm[:], op=ALU.add)
                nc.gpsimd.dma_start(out_d.ap()[ib * 128:(ib + 1) * 128, :],
                                    o[:])

            with tc.tile_pool(name="scores", bufs=2, space="PSUM") as scp, \
                 tc.tile_pool(name="wpl", bufs=12) as wpool, \
                 tc.tile_pool(name="ep", bufs=2, space="PSUM") as epp:
                for ib in range(NB):
                    emit_block(ib, ib - 1 if ib >= 1 else None)
                # tail: last block's softmax + epilogue
                lb = NB - 1
                pb = banks[lb]
                nc.scalar.activation(ee[lb][:], pb[:, :], AF.Exp,
                                     bias=0.0, accum_out=zsum[lb][:])
                nc.vector.reciprocal(rz[lb][:], zsum[lb][:])
                eT = sb.tile([128, 128 * NB], BF16, tag=f"eT{lb}",
                             name=f"eT{lb}")
                for jb in range(NB):
                    t = epp.tile([128, 128], BF16, tag="et")
                    nc.tensor.transpose(
                        t[:], ee[lb][:, jb * 128:(jb + 1) * 128], id128b[:])
                    nc.scalar.copy(eT[:, jb * 128:(jb + 1) * 128], t[:])
                acc = epp.tile([128, D], F32, tag="acc", name=f"acc{lb}",
                               bufs=1)
                accs[lb] = acc
                for jb in range(NB):
                    nc.tensor.matmul(
                        acc[:], eT[:, jb * 128:(jb + 1) * 128],
                        h_hi[:, jb * D:(jb + 1) * D],
                        start=(jb == 0), stop=False)
                    nc.tensor.matmul(
                        acc[:], eT[:, jb * 128:(jb + 1) * 128],
                        h_lo[:, jb * D:(jb + 1) * D],
                        start=False, stop=(jb == NB - 1))
                emit_out(lb)

    nc.compile()
    return nc


def kernel(x, W_proj, b_proj, W_cat_weight, W_cat_bias, a, bias_param):
    global last_results
    x = np.asarray(x, dtype=np.float32)
    W_proj = np.asarray(W_proj, dtype=np.float32)
    b_proj = np.asarray(b_proj, dtype=np.float32)
    W_cat_weight = np.asarray(W_cat_weight, dtype=np.float32)
    W_cat_bias = np.asarray(W_cat_bias, dtype=np.float32)
    a = np.asarray(a, dtype=np.float32)
    bias_param = np.asarray(bias_param, dtype=np.float32)

    W1 = W_cat_weight[:, :, :D]
    W2 = W_cat_weight[:, :, D:]

    use_bias_param = bool(np.any(bias_param))
    key = (use_bias_param,)
    if key not in _cache:
        _cache[key] = _build(*key)
    nc = _cache[key]

    id128 = np.eye(128, dtype=np.float32)
    s16 = None  # per-core below

    in_maps = []
    for c in range(8):
        b, hh = divmod(c, H)
        ah = a[hh]
        s = np.sign(ah).astype(np.float32)
        abs_a = np.abs(ah)
        Wp = W_proj[hh]
        bp = b_proj[hh]
        bc = W_cat_bias[hh]
        M1 = -0.8 * (abs_a[:, None] * W1[hh]) @ Wp.T     # [E, FIN]
        c1 = -0.8 * abs_a * (W1[hh] @ bp)                # [E]
        M2 = -0.8 * (abs_a[:, None] * W2[hh]) @ Wp.T     # [E, FIN]
        c2 = -0.8 * abs_a * (W2[hh] @ bp + bc)           # [E]
        mmt = np.concatenate([M1.T, M2.T], axis=1)       # [FIN, 2E]
        pks = np.zeros((128, 3), dtype=np.float32)
        pks[0:E, 0] = c1
        pks[0:E, 1] = c2
        pks[0:E, 2] = -1.25 * s
        s16 = np.zeros((128, 512), dtype=np.float32)
        for m in range(16):
            s16[0:E, 32 * m + 2 * m] = s
            s16[E:128, 32 * m + 2 * m + 1] = s
        m = {
            "x": np.ascontiguousarray(x[b]),
            "id128": id128,
            "mm": np.ascontiguousarray(mmt).astype(BF),
            "pks": pks,
            "wproj": np.ascontiguousarray(Wp),
            "s16": s16.astype(BF),
            "id128b": id128.astype(BF),
        }
        if use_bias_param:
            m["biasprm"] = np.tile(bias_param[None, hh * D:(hh + 1) * D],
                                   (128, 1)).astype(np.float32)
        in_maps.append(m)

    res = run_bass_kernel_spmd(nc, in_maps, core_ids=list(range(8)))
    last_results = res

    out = np.empty((B, N, H * D), dtype=np.float32)
    for c in range(8):
        b, hh = divmod(c, H)
        out[b, :, hh * D:(hh + 1) * D] = res.results[c]["out"]
    return out


# revision 4
# speedup vs baseline: 1.0472x; 1.0055x over previous
"""GATv2 Bass kernel v2 for Trainium2, 8 NeuronCores.

Problem: B=2, N=512, FIN=128, H=4, D=64 GATv2 attention (dense graph).
Sharding: one (batch, head) pair per core (B*H = 8 = n_cores).

Math per (b, h):
  h[n] = x[n] @ Wp + bp
  zi[n,e] = h[n]·W1[e]; zj[n,e] = h[n]·W2[e]; v_ije = zi[i,e]+zj[j,e]+bc[e]
  score[i,j] = sum_e a_e lrelu(v); attn = softmax_j; out = attn @ h

ReLU decomposition (v2): lrelu(v) = v + 0.8 ReLU(-v), so
  score[i,j] = A_i + B_j + sum_e s_e ReLU(u_i(e) + y_j(e))
  u_i(e) = -0.8|a_e| zi[i,e] (+bias), y_j(e) = -0.8|a_e| (zj[j,e]+bc_e)
  s_e = sign(a_e); A_i constant per row -> drops in softmax.
Kernel: e lives on PARTITIONS. Ydup [128,512] = yT stacked twice (bf16).
Per row-pair (2 rows per 128 partitions): W = ReLU(Ydup + u-col) via ONE
fused DVE tensor_scalar (add,max) at 2-4x rate, or ACT activation(Relu,
bias). PE contracts with +-1 stationaries [128,32] (16 pairs accumulate
into one 32-partition stripe of the score bank; tile_position picks the
stripe). B_j added exactly via a [2,128]x[2,512] accumulating matmul with
B split hi/lo in bf16. Softmax reads scores straight from PSUM; epilogue
(attn @ h with h split hi/lo bf16) as in v1. Softmax/epilogue emission is
deferred by one i-block so the strict-FIFO DVE/ACT queues never stall.
"""

import numpy as np
import ml_dtypes

import concourse.bacc as bacc
import concourse.mybir as mybir
import concourse.tile as tile
from concourse.bass_utils import run_bass_kernel_spmd

F32 = mybir.dt.float32
BF16 = mybir.dt.bfloat16
BF = ml_dtypes.bfloat16

B, N, FIN, H, D = 2, 512, 128, 4, 64
NEG_SLOPE = 0.2
E = D
NB = N // 128

last_results = None

_cache = {}


def _build(use_bias_param):
    nc = bacc.Bacc("TRN2", target_bir_lowering=False, debug=False,
                   num_devices=8)

    x_d = nc.dram_tensor("x", [N, FIN], F32, kind="ExternalInput")
    id128_d = nc.dram_tensor("id128", [128, 128], F32, kind="ExternalInput")
    mm_d = nc.dram_tensor("mm", [128, 2 * E], BF16, kind="ExternalInput")
    pks_d = nc.dram_tensor("pks", [128, 3], F32, kind="ExternalInput")
    wp_d = nc.dram_tensor("wproj", [FIN, D], F32, kind="ExternalInput")
    s16_d = nc.dram_tensor("s16", [128, 512], BF16, kind="ExternalInput")
    id128b_d = nc.dram_tensor("id128b", [128, 128], BF16,
                              kind="ExternalInput")
    if use_bias_param:
        bprm_d = nc.dram_tensor("biasprm", [128, D], F32,
                                kind="ExternalInput")
    out_d = nc.dram_tensor("out", [N, D], F32, kind="ExternalOutput")

    AF = mybir.ActivationFunctionType
    ALU = mybir.AluOpType
    AX = mybir.AxisListType

    with tile.TileContext(nc) as tc:
        with tc.tile_pool(name="sb", bufs=1) as sb:
            xb = sb.tile([128, NB * 128], F32)
            xT = sb.tile([128, N], F32)
            xTb = sb.tile([128, N], BF16)
            id128 = sb.tile([128, 128], F32)
            mmt = sb.tile([128, 2 * E], BF16)
            pks = sb.tile([128, 3], F32)
            sBb = sb.tile([E, 1], BF16)
            s16 = sb.tile([128, 512], BF16)
            wp = sb.tile([FIN, D], F32)
            id128b = sb.tile([128, 128], BF16)
            ydup = sb.tile([128, N], BF16)
            zidup = sb.tile([128, NB * 64], F32)
            yTsb = sb.tile([E, N], BF16)
            h_hi = sb.tile([128, NB * (D + 1)], BF16)
            h_lo = sb.tile([128, NB * (D + 1)], BF16)
            h_f = sb.tile([128, NB * D], F32)
            bpf = sb.tile([1, N], F32)
            expBc = sb.tile([128, NB], F32)
            ee = [sb.tile([128, N], BF16, tag=f"ee{i}", name=f"ee{i}")
                  for i in range(NB)]
            rowmax = [sb.tile([128, 1], F32, tag=f"rm{i}", name=f"rm{i}")
                      for i in range(NB)]
            negm = [sb.tile([128, 1], F32, tag=f"nm{i}", name=f"nm{i}")
                    for i in range(NB)]
            zsum = [sb.tile([128, 1], F32, tag=f"zs{i}", name=f"zs{i}")
                    for i in range(NB)]
            rz = [sb.tile([128, 1], F32, tag=f"rz{i}", name=f"rz{i}")
                  for i in range(NB)]
            if use_bias_param:
                bprm = sb.tile([128, D], F32)

            # ---------- input DMAs: spread across engine DGEs ----------
            for nb in range(NB):
                nc.sync.dma_start(
                    xb[:, nb * 128:(nb + 1) * 128],
                    x_d.ap()[nb * 128:(nb + 1) * 128, :])
            nc.scalar.dma_start(id128[:, 0:64], id128_d.ap()[:, 0:64])
            nc.scalar.dma_start(id128[:, 64:128], id128_d.ap()[:, 64:128])
            nc.gpsimd.dma_start(s16[:, 0:256], s16_d.ap()[:, 0:256])
            nc.gpsimd.dma_start(s16[:, 256:512], s16_d.ap()[:, 256:512])
            nc.scalar.dma_start(mmt[:], mm_d.ap())
            nc.scalar.dma_start(pks[:], pks_d.ap())
            nc.gpsimd.dma_start(wp[:], wp_d.ap())
            nc.gpsimd.dma_start(id128b[:], id128b_d.ap())
            if use_bias_param:
                nc.scalar.dma_start(bprm[:], bprm_d.ap())


            # ---------- prep ----------
            with tc.tile_pool(name="pp", bufs=4, space="PSUM") as pp:
                for nb in range(NB):
                    t = pp.tile([128, 512], F32, tag="t")
                    nc.tensor.transpose(t[:, 0:128],
                                        xb[:, nb * 128:(nb + 1) * 128],
                                        id128[:])
                    nc.scalar.copy(xT[:, nb * 128:(nb + 1) * 128],
                                   t[:, 0:128])
                    nc.vector.tensor_copy(xTb[:, nb * 128:(nb + 1) * 128],
                                          t[:, 0:128])
                nc.scalar.copy(sBb[:], pks[0:E, 2:3])
                # yT = M2 @ xT + c2 (bf16 mm) FIRST: it gates ydup -> maps
                yt = pp.tile([128, 512], F32, tag="t")
                nc.tensor.matmul(yt[0:E, :], mmt[:, E:2 * E], xTb[:],
                                 start=True, stop=True)
                # uT = M1 @ xT + c1 (bf16 mm), scattered into zidup (f32)
                ut = pp.tile([128, 512], F32, tag="t")
                nc.tensor.matmul(ut[0:E, :], mmt[:, 0:E], xTb[:],
                                 start=True, stop=True)
                # DVE: zidup scatter (fused add of c1) straight from PSUM
                utv = ut[0:E, :].rearrange("p (b q t) -> p b q t",
                                           b=NB, q=64, t=2)
                zde = zidup[0:E, :].rearrange("p (b q t) -> p b q t",
                                              b=NB, q=64, t=1)
                zdo = zidup[E:128, :].rearrange("p (b q t) -> p b q t",
                                                b=NB, q=64, t=1)
                nc.vector.tensor_scalar(zde[:, :, :, :], utv[:, :, :, 0:1],
                                        pks[0:E, 0:1], None, op0=ALU.add)
                nc.vector.tensor_scalar(zdo[:, :, :, :], utv[:, :, :, 1:2],
                                        pks[0:E, 0:1], None, op0=ALU.add)
                # ACT: ydup (gates the relu maps), then yTsb / b2
                nc.scalar.activation(ydup[0:E, :], yt[0:E, :], AF.Identity,
                                     bias=pks[0:E, 1:2])
                nc.scalar.activation(ydup[E:128, :], yt[0:E, :], AF.Identity,
                                     bias=pks[0:E, 1:2])
                nc.scalar.activation(yTsb[:], yt[0:E, :], AF.Identity,
                                     bias=pks[0:E, 1:2])

            # ---------- main: per i-block scores, softmax/epilogue of the
            # previous block interleaved at the HEAD of each block so the
            # strict-FIFO ACT/DVE queues never sit behind a full block of
            # relu maps ----------
            banks = []
            accs = {}
            eTs = {}
            hts = []
            bps = []

            def emit_map(ib, q, on_act):
                w = wpool.tile([128, N], BF16, tag="w")
                col = ib * 64 + q
                if on_act:
                    with nc.allow_low_precision(reason="bf16 relu"):
                        nc.scalar.activation(w[:], ydup[:], AF.Relu,
                                             bias=zidup[:, col:col + 1])
                else:
                    with nc.allow_low_precision(reason="bf16 relu"):
                        nc.vector.tensor_scalar(
                            w[:], ydup[:], zidup[:, col:col + 1],
                            0.0, op0=ALU.add, op1=ALU.max)
                st, m = divmod(q, 16)
                nc.tensor.matmul(
                    bank[32 * st:32 * st + 32, :],
                    s16[:, 32 * m:32 * m + 32], w[:],
                    start=(m == 0), stop=(m == 15),
                    tile_position=(0, 32 * st),
                    skip_group_check=True)

            def act_map(q):
                # ACT handles every 4th map from q=15 on; the head of each
                # block is DVE-only so ACT can run the previous block's
                # exp/epilogue without stalling the PE map stream
                return q >= 12 and q % 4 == 3

            def emit_block(ib, prev):
                global bank
                bank = scp.tile([128, N], F32, tag="bank", name=f"bank{ib}")
                banks.append(bank)
                for q in range(64):
                    emit_map(ib, q, act_map(q))
                    if prev is None:
                        if q == 2:
                            hall = epp.tile([128, NB * D], F32, tag="hall",
                                            bufs=1)
                            hts.append(hall)
                            for nb in range(NB):
                                nc.tensor.matmul(
                                    hall[:, nb * D:(nb + 1) * D],
                                    xT[:, nb * 128:(nb + 1) * 128],
                                    wp[:], start=True, stop=True,
                                    skip_group_check=True)
                        elif q == 3:
                            bp_ = epp.tile([128, 512], F32, tag="bp",
                                           bufs=1)
                            bps.append(bp_)
                            nc.tensor.matmul(bp_[0:1, :], sBb[:], yTsb[:],
                                             start=True, stop=True)
                        elif q == 4:
                            nc.scalar.copy(bpf[:], bps[0][0:1, :])
                        elif q == 5:
                            # transpose B row chunks into columns [128, NB]
                            bt = epp.tile([128, NB], F32, tag="bt", bufs=1)
                            bps.append(bt)
                            for c in range(NB):
                                nc.tensor.transpose(
                                    bt[:, c:c + 1],
                                    bpf[0:1, c * 128:(c + 1) * 128],
                                    id128[0:1, 0:1])
                        elif q == 7:
                            nc.scalar.activation(expBc[:], bps[1][:, :],
                                                 AF.Exp, bias=0.0)
                        elif q == 9:
                            for nb in range(NB):
                                nc.scalar.activation(
                                    h_hi[:, nb * 65:nb * 65 + D],
                                    hts[0][:, nb * D:(nb + 1) * D],
                                    AF.Copy, bias=0.0,
                                    scale=expBc[:, nb:nb + 1])
                                nc.scalar.activation(
                                    h_f[:, nb * D:(nb + 1) * D],
                                    hts[0][:, nb * D:(nb + 1) * D],
                                    AF.Copy, bias=0.0,
                                    scale=expBc[:, nb:nb + 1])
                        elif q == 11:
                            for nb in range(NB):
                                nc.scalar.copy(h_hi[:, nb * 65 + D:
                                                    nb * 65 + D + 1],
                                               expBc[:, nb:nb + 1])
                        elif q == 24:
                            for nb in range(NB):
                                nc.vector.tensor_tensor(
                                    h_lo[:, nb * 65:nb * 65 + D],
                                    h_f[:, nb * D:(nb + 1) * D],
                                    h_hi[:, nb * 65:nb * 65 + D],
                                    op=ALU.subtract)
                            for nb in range(NB):
                                nc.vector.tensor_tensor(
                                    h_lo[:, nb * 65 + D:nb * 65 + D + 1],
                                    expBc[:, nb:nb + 1],
                                    h_hi[:, nb * 65 + D:nb * 65 + D + 1],
                                    op=ALU.subtract)
                        continue
                    if q == 7:
                        # ACT exp for prev block (uncentered: scores bounded)
                        pb = banks[prev]
                        nc.scalar.activation(ee[prev][:], pb[:, :], AF.Exp,
                                             bias=0.0)
                    elif q == 9:
                        eT = sb.tile([128, 128 * NB], BF16, tag=f"eT{prev}",
                                     name=f"eT{prev}")
                        eTs[prev] = eT
                        for jb in range(NB):
                            t = epp.tile([128, 128], BF16, tag="et")
                            nc.tensor.transpose(
                                t[:], ee[prev][:, jb * 128:(jb + 1) * 128],
                                id128b[:])
                            nc.scalar.copy(
                                eT[:, jb * 128:(jb + 1) * 128], t[:])
                    elif q == 13:
                        eT = eTs[prev]
                        acc = epp.tile([128, D + 1], F32, tag="acc",
                                       name=f"acc{prev}", bufs=1)
                        accs[prev] = acc
                        for jb in range(NB):
                            nc.tensor.matmul(
                                acc[:], eT[:, jb * 128:(jb + 1) * 128],
                                h_hi[:, jb * 65:(jb + 1) * 65],
                                start=(jb == 0), stop=False)
                            nc.tensor.matmul(
                                acc[:], eT[:, jb * 128:(jb + 1) * 128],
                                h_lo[:, jb * 65:(jb + 1) * 65],
                                start=False, stop=(jb == NB - 1))
                    elif q == 15:
                        nc.vector.reciprocal(rz[prev][:],
                                             accs[prev][:, D:D + 1])
                    elif q == 17:
                        emit_out(prev)

            def emit_out(ib):
                o = sb.tile([128, D], F32, tag=f"o{ib}", name=f"o{ib}")
                nc.scalar.activation(o[:], accs[ib][:, 0:D], AF.Copy,
                                     bias=0.0, scale=rz[ib][:, 0:1])
                if use_bias_param:
                    nc.vector.tensor_tensor(o[:], o[:], bprm[:], op=ALU.add)
                nc.gpsimd.dma_start(out_d.ap()[ib * 128:(ib + 1) * 128, :],
                                    o[:])

            with tc.tile_pool(name="scores", bufs=2, space="PSUM") as scp, \
                 tc.tile_pool(name="wpl", bufs=12) as wpool, \
                 tc.tile_pool(name="ep", bufs=2, space="PSUM") as epp:
                for ib in range(NB):
                    emit_block(ib, ib - 1 if ib >= 1 else None)
                # tail: last block's softmax + epilogue
                lb = NB - 1
                pb = banks[lb]
                nc.scalar.activation(ee[lb][:], pb[:, :], AF.Exp,
                                     bias=0.0)
                eT = sb.tile([128, 128 * NB], BF16, tag=f"eT{lb}",
                             name=f"eT{lb}")
                for jb in range(NB):
                    t = epp.tile([128, 128], BF16, tag="et")
                    nc.tensor.transpose(
                        t[:], ee[lb][:, jb * 128:(jb + 1) * 128], id128b[:])
                    nc.scalar.copy(eT[:, jb * 128:(jb + 1) * 128], t[:])
                acc = epp.tile([128, D + 1], F32, tag="acc", name=f"acc{lb}",
                               bufs=1)
                accs[lb] = acc
                for jb in range(NB):
                    nc.tensor.matmul(
                        acc[:], eT[:, jb * 128:(jb + 1) * 128],
                        h_hi[:, jb * 65:(jb + 1) * 65],
                        start=(jb == 0), stop=False)
                    nc.tensor.matmul(
                        acc[:], eT[:, jb * 128:(jb + 1) * 128],
                        h_lo[:, jb * 65:(jb + 1) * 65],
                        start=False, stop=(jb == NB - 1))
                nc.vector.reciprocal(rz[lb][:], acc[:, D:D + 1])
                emit_out(lb)

    nc.compile()
    return nc


def kernel(x, W_proj, b_proj, W_cat_weight, W_cat_bias, a, bias_param):
    global last_results
    x = np.asarray(x, dtype=np.float32)
    W_proj = np.asarray(W_proj, dtype=np.float32)
    b_proj = np.asarray(b_proj, dtype=np.float32)
    W_cat_weight = np.asarray(W_cat_weight, dtype=np.float32)
    W_cat_bias = np.asarray(W_cat_bias, dtype=np.float32)
    a = np.asarray(a, dtype=np.float32)
    bias_param = np.asarray(bias_param, dtype=np.float32)

    W1 = W_cat_weight[:, :, :D]
    W2 = W_cat_weight[:, :, D:]

    use_bias_param = bool(np.any(bias_param))
    key = (use_bias_param,)
    if key not in _cache:
        _cache[key] = _build(*key)
    nc = _cache[key]

    id128 = np.eye(128, dtype=np.float32)
    s16 = None  # per-core below

    in_maps = []
    for c in range(8):
        b, hh = divmod(c, H)
        ah = a[hh]
        s = np.sign(ah).astype(np.float32)
        abs_a = np.abs(ah)
        Wp = W_proj[hh]
        bp = b_proj[hh]
        bc = W_cat_bias[hh]
        M1 = -0.8 * (abs_a[:, None] * W1[hh]) @ Wp.T     # [E, FIN]
        c1 = -0.8 * abs_a * (W1[hh] @ bp)                # [E]
        M2 = -0.8 * (abs_a[:, None] * W2[hh]) @ Wp.T     # [E, FIN]
        c2 = -0.8 * abs_a * (W2[hh] @ bp + bc)           # [E]
        mmt = np.concatenate([M1.T, M2.T], axis=1)       # [FIN, 2E]
        pks = np.zeros((128, 3), dtype=np.float32)
        pks[0:E, 0] = c1
        pks[0:E, 1] = c2
        pks[0:E, 2] = -1.25 * s
        s16 = np.zeros((128, 512), dtype=np.float32)
        for m in range(16):
            s16[0:E, 32 * m + 2 * m] = s
            s16[E:128, 32 * m + 2 * m + 1] = s
        m = {
            "x": np.ascontiguousarray(x[b]),
            "id128": id128,
            "mm": np.ascontiguousarray(mmt).astype(BF),
            "pks": pks,
            "wproj": np.ascontiguousarray(Wp),
            "s16": s16.astype(BF),
            "id128b": id128.astype(BF),
        }
        if use_bias_param:
            m["biasprm"] = np.tile(bias_param[None, hh * D:(hh + 1) * D],
                                   (128, 1)).astype(np.float32)
        in_maps.append(m)

    res = run_bass_kernel_spmd(nc, in_maps, core_ids=list(range(8)))
    last_results = res

    out = np.empty((B, N, H * D), dtype=np.float32)
    for c in range(8):
        b, hh = divmod(c, H)
        out[b, :, hh * D:(hh + 1) * D] = res.results[c]["out"]
    return out
